# revision 4
# baseline (speedup 1.0000x reference)
"""Trainium2 Bass kernel for nn_Attention_40561671144003.

Head-parallel sharding: 8 heads -> 8 NeuronCores, one head per core.
Each core computes its head's q/k/v projections (reading the full
replicated activations), full-sequence attention for that head, and a
partial output projection.  The host sums the 8 partial projections
(the "all-reduce after proj" step) and overlays each core's
vflat-contribution rows.

Math per core (head h), all derived on host in check against reference:
  xT   = concat(query, value, axis=-1).T                      [1024, 4096]
  qT/kT/vT = w_{q,k,v}[h].T.T @ xT                            [64, 4096]
  sT   = kT.T @ qT        (scores, transposed: [m, n])
  pT   = exp(sT / 8)      (no max-subtraction: |s/8| <= ~9)
  o_aug= pT.T @ [v | 1]   -> o = o_aug[:, :64] / o_aug[:, 64] [n, 64]
  ypart  = o @ w_proj[:, h*64:(h+1)*64].T                     [4096, 1024]
  yvout  = v.reshape(512, 512) @ w_proj[:, 512:].T + b_proj   [512, 1024]
  host: y = sum_h ypart_h;  y[h*512:(h+1)*512] += yvout_h
"""

from contextlib import ExitStack

import numpy as np

import concourse.bass as bass
import concourse.bacc as bacc
import concourse.tile as tile
from concourse import mybir
from concourse.bass_utils import run_bass_kernel_spmd
from concourse.masks import make_identity

FP32 = mybir.dt.float32
BF16 = mybir.dt.bfloat16
Exp = mybir.ActivationFunctionType.Exp

N = 4096          # sequence length
C = 512           # channels
H = 8             # heads
D = 64            # head dim
SB = 512          # sequence block (n-block and qkv block)
NSB = N // SB     # 8
MT = N // 128     # 32 m-tiles of 128
GM = 2            # m-tiles per score/exp group
NG = MT // GM     # 16 groups per n-block
SCALE = D ** -0.5


def _emit(ctx, tc, nc, io):
    xT, wqkT, wvT, w1, w2, bvec, ypart, yvout = io

    consts = ctx.enter_context(tc.tile_pool(name="consts", bufs=1))
    persist = ctx.enter_context(tc.tile_pool(name="persist", bufs=1))
    xpool = ctx.enter_context(tc.tile_pool(name="xpool", bufs=10))
    ppool = ctx.enter_context(tc.tile_pool(name="ppool", bufs=3))
    otpool = ctx.enter_context(tc.tile_pool(name="otpool", bufs=6))
    ypool = ctx.enter_context(tc.tile_pool(name="ypool", bufs=3))
    smalls = ctx.enter_context(tc.tile_pool(name="smalls", bufs=6))
    psA = ctx.enter_context(tc.tile_pool(name="psA", bufs=2, space="PSUM"))
    psO = ctx.enter_context(tc.tile_pool(name="psO", bufs=2, space="PSUM"))
    psS = ctx.enter_context(tc.tile_pool(name="psS", bufs=2, space="PSUM"))

    # ---- constants ----
    wqk_sb = consts.tile([128, 8, 128], FP32, tag="wqk")
    nc.sync.dma_start(out=wqk_sb, in_=wqkT.rearrange("(t p) m -> p t m", p=128))
    wv_sb = consts.tile([128, 8, 64], FP32, tag="wv")
    nc.sync.dma_start(out=wv_sb, in_=wvT.rearrange("(t p) m -> p t m", p=128))
    w1_sb = consts.tile([64, 1024], FP32, tag="w1")
    nc.sync.dma_start(out=w1_sb, in_=w1[:, :])
    w2_sb = consts.tile([128, 4, 1024], FP32, tag="w2")
    nc.sync.dma_start(out=w2_sb, in_=w2.rearrange("(t p) j -> p t j", p=128))
    bv_sb = consts.tile([1, 1024], FP32, tag="bv")
    nc.sync.dma_start(out=bv_sb, in_=bvec[:, :])
    ident = consts.tile([128, 128], FP32, tag="ident")
    make_identity(nc, ident)
    ones_sb = consts.tile([1, 128], FP32, tag="ones")
    nc.vector.memset(ones_sb, 1.0)

    # ---- phase 1: qkv projections ----
    qT = [persist.tile([64, SB], FP32, tag=f"qT{i}", name=f"qT{i}") for i in range(NSB)]
    kT = [persist.tile([64, SB], FP32, tag=f"kT{i}", name=f"kT{i}") for i in range(NSB)]
    vT = [persist.tile([64, SB], FP32, tag=f"vT{i}", name=f"vT{i}") for i in range(NSB)]
    vaug = [persist.tile([128, 65], BF16, tag=f"va{i}", name=f"va{i}") for i in range(MT)]

    for sblk in range(NSB):
        xts = []
        for c in range(8):
            xt = xpool.tile([128, SB], FP32, name=f"xt{sblk}_{c}", tag="xt")
            nc.sync.dma_start(
                out=xt, in_=xT[c * 128:(c + 1) * 128, sblk * SB:(sblk + 1) * SB]
            )
            xts.append(xt)
        qk_ps = psS.tile([128, SB], FP32, name=f"qkps{sblk}", tag="sp")
        for c in range(8):
            nc.tensor.matmul(
                qk_ps, lhsT=wqk_sb[:, c, :], rhs=xts[c], start=(c == 0), stop=(c == 7)
            )
        nc.vector.tensor_copy(qT[sblk], qk_ps[0:64, :])
        nc.vector.tensor_copy(kT[sblk], qk_ps[64:128, :])
        v_ps = psS.tile([128, SB], FP32, name=f"vps{sblk}", tag="sp")
        for c in range(8):
            nc.tensor.matmul(
                v_ps[0:64, :], lhsT=wv_sb[:, c, :], rhs=xts[c],
                start=(c == 0), stop=(c == 7),
            )
        nc.vector.tensor_copy(vT[sblk], v_ps[0:64, :])
        # v natural layout [m, d] (+ ones column) for the ov matmul
        for j in range(4):
            mt = sblk * 4 + j
            tr_ps = psS.tile([128, 64], FP32, name=f"vtr{mt}", tag="sp")
            nc.tensor.transpose(
                tr_ps, vT[sblk][:, j * 128:(j + 1) * 128], ident[0:64, 0:64]
            )
            nc.vector.tensor_copy(vaug[mt][:, 0:64], tr_ps)
            nc.vector.memset(vaug[mt][:, 64:65], 1.0)

    # vflatT[c2, r] = v[8r + c2//64, c2%64] for the proj's vflat rows
    vflatT = [
        persist.tile([128, 128], FP32, tag=f"vf{i}", name=f"vf{i}") for i in range(16)
    ]
    for k2 in range(4):
        for rt in range(4):
            vf = vflatT[k2 * 4 + rt]
            for s01 in range(2):
                for half in range(2):
                    t = 2 * rt + half
                    src = vT[t].rearrange("p (r s) -> p s r", s=8)
                    nc.vector.tensor_copy(
                        vf[64 * s01:64 * s01 + 64, 64 * half:64 * half + 64],
                        src[:, 2 * k2 + s01, :],
                    )

    # ---- attention + output projection, software-pipelined ----
    groups = [(nb, g) for nb in range(NSB) for g in range(NG)]
    s_ps = [None] * len(groups)
    p_sb = [None] * len(groups)

    def emit_s(i):
        nb, g = groups[i]
        ps = psA.tile([128, GM * SB], FP32, name=f"sps{nb}_{g}", tag="sps")
        for j in range(GM):
            mt = g * GM + j
            t, off = mt // 4, (mt % 4) * 128
            nc.tensor.matmul(
                ps[:, j * SB:(j + 1) * SB],
                lhsT=kT[t][:, off:off + 128], rhs=qT[nb],
                start=True, stop=True,
            )
        s_ps[i] = ps

    def emit_exp(i):
        nb, g = groups[i]
        pt = ppool.tile([128, GM * SB], BF16, name=f"pt{nb}_{g}", tag="pt")
        nc.scalar.activation(pt, s_ps[i], Exp, scale=SCALE)
        p_sb[i] = pt

    oaug = [None] * NSB

    def emit_ov(i):
        nb, g = groups[i]
        if g == 0:
            oaug[nb] = psO.tile([128, SB], FP32, name=f"oaug{nb}", tag="oaug")
        pt = p_sb[i]
        for j in range(GM):
            mt = g * GM + j
            for nt in range(4):
                # One accumulation group per PSUM bank: start marks the whole
                # 2KB zero-region pending, so only the first MM of the bank
                # starts and only the very last stops; each slice's first
                # write overwrites via the pending-zero bits.
                nc.tensor.matmul(
                    oaug[nb][:, nt * 128:nt * 128 + 65],
                    lhsT=pt[:, j * SB + nt * 128:j * SB + nt * 128 + 128],
                    rhs=vaug[mt],
                    start=(mt == 0 and nt == 0), stop=(mt == MT - 1 and nt == 3),
                )
        p_sb[i] = None
        s_ps[i] = None

    def emit_tail(nb):
        # normalize o, transpose, project (yo), write partial rows
        for nt in range(4):
            sl = oaug[nb][:, nt * 128:nt * 128 + 65]
            rec = smalls.tile([128, 1], FP32, name=f"rec{nb}_{nt}", tag="rec")
            nc.vector.reciprocal(rec, sl[:, 64:65])
            onrm = smalls.tile([128, 64], FP32, name=f"on{nb}_{nt}", tag="onrm")
            nc.vector.tensor_scalar_mul(onrm, sl[:, 0:64], rec)
            tr_ps = psS.tile([64, 128], FP32, name=f"otr{nb}_{nt}", tag="sp")
            nc.tensor.transpose(tr_ps, onrm, ident)
            oTt = otpool.tile([64, 128], FP32, name=f"oT{nb}_{nt}", tag="oT")
            nc.vector.tensor_copy(oTt, tr_ps)
            ystage = ypool.tile([128, 1024], FP32, name=f"yst{nb}_{nt}", tag="yst")
            for half in range(2):
                yps = psS.tile([128, SB], FP32, name=f"yops{nb}_{nt}_{half}", tag="sp")
                nc.tensor.matmul(
                    yps, lhsT=oTt, rhs=w1_sb[:, half * 512:(half + 1) * 512],
                    start=True, stop=True,
                )
                nc.vector.tensor_copy(ystage[:, half * 512:(half + 1) * 512], yps)
            row = (nb * 4 + nt) * 128
            nc.sync.dma_start(out=ypart[row:row + 128, :], in_=ystage)
        oaug[nb] = None

    def emit_yv():
        for rt in range(4):
            yvstage = ypool.tile([128, 1024], FP32, name=f"yvst{rt}", tag="yst")
            for half in range(2):
                yps = psS.tile([128, SB], FP32, name=f"yvps{rt}_{half}", tag="sp")
                for k2 in range(4):
                    nc.tensor.matmul(
                        yps, lhsT=vflatT[k2 * 4 + rt],
                        rhs=w2_sb[:, k2, half * 512:(half + 1) * 512],
                        start=(k2 == 0), stop=False,
                    )
                nc.tensor.matmul(
                    yps, lhsT=ones_sb,
                    rhs=bv_sb[:, half * 512:(half + 1) * 512],
                    start=False, stop=True,
                )
                nc.vector.tensor_copy(yvstage[:, half * 512:(half + 1) * 512], yps)
            nc.sync.dma_start(out=yvout[rt * 128:(rt + 1) * 128, :], in_=yvstage)

    # lookahead-2 pipeline: scores for group i+2 are emitted (PE program
    # order) before the ov of group i, so the PE never sits behind the
    # ACT exp of the group it is about to consume.
    emit_s(0)
    emit_s(1)
    for i in range(len(groups)):
        if i + 2 < len(groups):
            emit_s(i + 2)
        emit_exp(i)
        emit_ov(i)
        nb, g = groups[i]
        if g == NG - 1:
            emit_tail(nb)
            if nb == 0:
                emit_yv()


def build_program():
    nc = bacc.Bacc()
    xT = nc.declare_dram_parameter("xT", [1024, N], FP32, isOutput=False)
    wqkT = nc.declare_dram_parameter("wqkT", [1024, 128], FP32, isOutput=False)
    wvT = nc.declare_dram_parameter("wvT", [1024, 64], FP32, isOutput=False)
    w1 = nc.declare_dram_parameter("w1", [64, 1024], FP32, isOutput=False)
    w2 = nc.declare_dram_parameter("w2", [512, 1024], FP32, isOutput=False)
    bvec = nc.declare_dram_parameter("bvec", [1, 1024], FP32, isOutput=False)
    ypart = nc.declare_dram_parameter("ypart", [N, 1024], FP32, isOutput=True)
    yvout = nc.declare_dram_parameter("yvout", [512, 1024], FP32, isOutput=True)
    io = (xT[:], wqkT[:], wvT[:], w1[:], w2[:], bvec[:], ypart[:], yvout[:])
    with tile.TileContext(nc) as tc:
        with ExitStack() as ctx:
            _emit(ctx, tc, nc, io)
    nc.compile()
    return nc


def make_in_maps(query, value, w_qkv, w_proj, b_proj):
    x = np.concatenate([query[0], value[0]], axis=1).astype(np.float32)
    xT = np.ascontiguousarray(x.T)
    w2 = np.ascontiguousarray(w_proj[:, 512:1024].T.astype(np.float32))
    bv = np.ascontiguousarray(b_proj[None, :].astype(np.float32))
    in_maps = []
    for h in range(H):
        wqk = np.ascontiguousarray(
            np.concatenate(
                [w_qkv[h * 64:(h + 1) * 64], w_qkv[512 + h * 64:512 + (h + 1) * 64]],
                axis=0,
            ).T.astype(np.float32)
        )
        wv = np.ascontiguousarray(
            w_qkv[1024 + h * 64:1024 + (h + 1) * 64].T.astype(np.float32)
        )
        w1 = np.ascontiguousarray(w_proj[:, h * 64:(h + 1) * 64].T.astype(np.float32))
        in_maps.append(
            {"xT": xT, "wqkT": wqk, "wvT": wv, "w1": w1, "w2": w2, "bvec": bv}
        )
    return in_maps


def combine_results(results):
    y = np.zeros((N, 1024), np.float32)
    for h in range(H):
        y += results[h]["ypart"]
    for h in range(H):
        y[h * 512:(h + 1) * 512] += results[h]["yvout"]
    y0 = np.ascontiguousarray(y[:, :512].reshape(1, N, 512))
    y1 = np.ascontiguousarray(y[:, 512:].reshape(1, N, 512))
    return y0, y1


_PROGRAM = None


def kernel(query, value, w_qkv, w_proj, b_proj, **_):
    global _PROGRAM
    if _PROGRAM is None:
        _PROGRAM = build_program()
    in_maps = make_in_maps(query, value, w_qkv, w_proj, b_proj)
    res = run_bass_kernel_spmd(_PROGRAM, in_maps, list(range(H)))
    return combine_results(res.results)
